# revision 16
# baseline (speedup 1.0000x reference)
"""Trainium2 Bass kernel for nn_BlockBlastValueNet1PmultikernelFlattenned.

Strategy (v10)
--------------
The network is 8 tiny conv branches over an 8x8 board followed by small MLPs.
Because the board has only 64 pixels, every conv branch (pad const 1.0 +
valid conv + bias) is an affine map of the 64 board values.  The whole net
folds into:

    y  = x @ W1 + c1                     # [B, NF]  (NF = 2944 padded)
    h  = Lrelu( Lrelu(y) @ W2' + b2 )    # per-branch first FC, block diagonal
    g1 = Lrelu( h @ W3 + b3 )            # branch second FC fused with fc1
    g2 = Lrelu( g1 @ W4 + b4 )           # fc2 (augmented with a ones column)
    out = g2 @ W5                        # fc3 (bias folded via augmentation)

Data-parallel over 8 NeuronCores (batch 65536 -> 8192/core), processed in
8 pair-iterations of 1024 samples.  Features live on SBUF partitions,
samples stream on the free dim.  All matmuls are fp16, 512-wide chunks.

Two hardware constraints shape the schedule:

1. PSUM evacuation is the floor: every y element must pass through the
   Scalar or Vector engine exactly once (~1 elem/cycle/lane each), so both
   engines must stay ~100% busy.  y tiles are split ~11/12 between Scalar
   (exact Lrelu ACT) and Vector (relu via tensor_scalar max; the 0.01*y
   leaky path of vector tiles folds into a small 64->128 matmul W12).
2. The PE HAM clock-gate measures *array activity*, not just busyness:
   half-array (K=64) matmuls count ~50% active and the PE gets re-throttled
   to 1.2 GHz during any K=64-dominated phase, doubling matmul time.  So
   step-1 runs as full K=128 matmuls: the input is [x; 1; zeros] and each
   step-1 weight tile carries the y-bias c1 in row 64 — same matmul cost
   (cost depends only on moving width), full activity, and the bias lands
   in PSUM for free so evacuations need no per-tile bias operand.

Schedule per 1024-sample pair: 23 "units" (one y tile each: two 512-wide
step-1 matmuls + one evacuation), with the step-2 column chains (four
32-column accumulation chains at tile_position (0,32g), one per branch
pair-group, K=128) emitted as chunk-major rounds between units, lagging
evacuation by ~2 units.  Chain steps that would wait on the last
evacuations are carried into the NEXT pair's units 0-3 so the in-order PE
queue never parks a flush while the evac engines starve.  The serial tail
(h -> g1 -> g2 -> out) pops at fixed units of the next pair.  A ~8us
warm-up block of dummy K=128 matmuls bridges the startup DMA window so
the HAM gate is open before real work starts.
"""

import numpy as np

# ---------------------------------------------------------------- constants
SPECS = [(1, 1, 1, 0, 0), (2, 2, 6, 1, 1), (3, 3, 8, 1, 1), (4, 4, 8, 2, 2),
         (5, 5, 16, 2, 2), (8, 8, 32, 0, 0), (1, 8, 4, 0, 0), (8, 1, 4, 0, 0)]
BOARD = 8
B_TOTAL = 65536
N_CORES = 8
BC = B_TOTAL // N_CORES          # 8192 samples per core
PAIR_N = 1024                    # samples per pair-iteration
CHUNK = 512                      # matmul moving width (1 psum bank fp32)
N_PAIRS = BC // PAIR_N           # 8

# column-group pairing of branches (2 branches x 16 h-outputs = 32 cols each).
# y rows are packed tightly in group order; a K-tile straddling a group
# boundary appears in BOTH groups' chains (the block-diagonal W2 already has
# zeros for the other group's columns).
GROUPS = [[4, 5], [3, 6], [2, 7], [1, 0]]
_BR_N = []
for kh, kw, fs, ph, pw in SPECS:
    _BR_N.append((BOARD + 2 * ph - kh + 1) * (BOARD + 2 * pw - kw + 1) * fs)
_NF_TRUE = sum(_BR_N)            # 2830
KT = -(-_NF_TRUE // 128)         # 23 K-tiles / M-tiles of y
NF = KT * 128                    # 2944 (last tile zero-padded)
_G_ROWS = []
_o = 0
for g in GROUPS:
    sz = sum(_BR_N[b] for b in g)
    _G_ROWS.append((_o, _o + sz))
    _o += sz
GROUP_TR = [(r0 // 128, -(-r1 // 128)) for r0, r1 in _G_ROWS]  # tile ranges

LRELU_NEG = 0.01

# evac engine split: these tiles go to the Vector engine (relu + fold path),
# the rest to Scalar (exact Lrelu ACT).  Units 0-3 are all-vector because at
# pair start the Scalar engine still runs the previous pair's h activation.
VEC_TILES = frozenset({0, 1, 2, 3, 5, 7, 9, 11, 13, 15, 17, 19})


# ---------------------------------------------------------------- host fold
def _fold_params(p):
    """Fold conv branches + MLPs into the dense pipeline weights (float64)."""
    n_of = _BR_N
    W1_of, c1_of = {}, {}
    for i, (kh, kw, fs, ph, pw) in enumerate(SPECS):
        Ho = BOARD + 2 * ph - kh + 1
        Wo = BOARD + 2 * pw - kw + 1
        cw = np.asarray(p[f"b{i}_cw"], np.float64)
        cb = np.asarray(p[f"b{i}_cb"], np.float64)
        W1 = np.zeros((64, n_of[i]))
        c1 = np.zeros((n_of[i],))
        for f in range(fs):
            for oh in range(Ho):
                for ow in range(Wo):
                    oi = (f * Ho + oh) * Wo + ow
                    c1[oi] += cb[f]
                    for u in range(kh):
                        for v in range(kw):
                            r, c = oh + u - ph, ow + v - pw
                            w = cw[f, 0, u, v]
                            if 0 <= r < 8 and 0 <= c < 8:
                                W1[r * 8 + c, oi] += w
                            else:
                                c1[oi] += w        # pad value is 1.0
        W1_of[i] = W1
        c1_of[i] = c1

    # K-layout: groups concatenated tightly; only the final tile is padded
    K_start = {}
    off = 0
    for g in GROUPS:
        for b in g:
            K_start[b] = off
            off += n_of[b]
    assert off == _NF_TRUE
    border = [b for g in GROUPS for b in g]       # h block order
    hpos = {b: j * 16 for j, b in enumerate(border)}

    W1p = np.zeros((64, NF))
    c1p = np.zeros((NF,))
    W2p = np.zeros((NF, 128))
    b2p = np.zeros((128,))
    for b in range(8):
        s, n, hp = K_start[b], n_of[b], hpos[b]
        W1p[:, s:s + n] = W1_of[b]
        c1p[s:s + n] = c1_of[b]
        W2p[s:s + n, hp:hp + 16] = np.asarray(p[f"b{b}_w1"], np.float64).T
        b2p[hp:hp + 16] = np.asarray(p[f"b{b}_b1"], np.float64)

    Wb = np.zeros((128, 64))
    bb = np.zeros((64,))
    for b in range(8):
        hp = hpos[b]
        Wb[hp:hp + 16, 8 * b:8 * b + 8] = np.asarray(p[f"b{b}_w2"], np.float64).T
        bb[8 * b:8 * b + 8] = np.asarray(p[f"b{b}_b2"], np.float64)
    fc_w1 = np.asarray(p["fc_w1"], np.float64)
    fc_b1 = np.asarray(p["fc_b1"], np.float64)
    W3 = Wb @ fc_w1.T
    b3 = bb @ fc_w1.T + fc_b1
    fc_w2 = np.asarray(p["fc_w2"], np.float64)
    fc_b2 = np.asarray(p["fc_b2"], np.float64)
    fc_w3 = np.asarray(p["fc_w3"], np.float64)
    fc_b3 = np.asarray(p["fc_b3"], np.float64)
    W4 = np.zeros((64, 17)); W4[:, :16] = fc_w2.T
    b4 = np.zeros((17,)); b4[:16] = fc_b2; b4[16] = 1.0
    W5 = np.zeros((17, 1)); W5[:16, 0] = fc_w3[0]; W5[16, 0] = fc_b3[0]

    # Vector-assigned tiles use relu + decomposition:
    #   Lrelu(v) = 0.01*v + 0.99*relu(v)
    # so their W2 rows are scaled by 0.99 and the 0.01*v path (affine in x)
    # folds into W12 / b2f.  Scalar-assigned tiles compute Lrelu exactly on
    # the ACT engine, so their W2 rows stay unscaled and contribute nothing
    # to the fold.
    vmask = np.zeros((NF, 1))
    for t in range(KT):
        if t in VEC_TILES:
            vmask[128 * t:128 * (t + 1)] = 1.0
    W2s = np.where(vmask > 0, (1.0 - LRELU_NEG) * W2p, W2p)
    W12 = LRELU_NEG * ((W1p * vmask[:, 0][None, :]) @ W2p)
    b2f = LRELU_NEG * ((c1p * vmask[:, 0]) @ W2p) + b2p

    f32 = np.float32
    f16 = np.float16
    dev = {}
    # step-1 weight tiles: full K=128 (input is [x; 1; zeros]); row 64
    # carries the y-bias c1 so the PSUM result is already biased.
    w1 = np.zeros((128, KT, 128), f16)
    for t in range(KT):
        w1[0:64, t, :] = W1p[:, 128 * t:128 * (t + 1)]
        w1[64, t, :] = c1p[128 * t:128 * (t + 1)]
    dev["w1"] = w1
    w2 = np.zeros((128, KT, 128), f16)
    for t in range(KT):
        w2[:, t, :] = W2s[128 * t:128 * (t + 1), :]
    dev["w2"] = w2
    dev["w12"] = W12.astype(f16)
    dev["b2f"] = b2f.reshape(128, 1).astype(f32)
    dev["w3"] = W3.astype(f16)
    dev["b3"] = b3.reshape(64, 1).astype(f32)
    dev["w4"] = W4.astype(f16)
    dev["b4"] = b4.reshape(17, 1).astype(f32)
    dev["w5"] = W5.astype(f16)
    return dev


# ---------------------------------------------------------------- device IR
def _build_nc(n_pairs=N_PAIRS):
    import concourse.mybir as mybir
    import concourse.tile as tile
    from concourse import bacc
    from contextlib import ExitStack

    dt = mybir.dt
    AF = mybir.ActivationFunctionType
    f32 = dt.float32
    f16 = dt.float16
    bc = n_pairs * PAIR_N

    nc = bacc.Bacc("TRN2", target_bir_lowering=False, debug=False,
                   num_devices=N_CORES)

    xx_d = nc.dram_tensor("xx", [128, bc], f16, kind="ExternalInput")
    w1_d = nc.dram_tensor("w1", [128, KT, 128], f16, kind="ExternalInput")
    w2_d = nc.dram_tensor("w2", [128, KT, 128], f16, kind="ExternalInput")
    w12_d = nc.dram_tensor("w12", [64, 128], f16, kind="ExternalInput")
    b2f_d = nc.dram_tensor("b2f", [128, 1], f32, kind="ExternalInput")
    w3_d = nc.dram_tensor("w3", [128, 64], f16, kind="ExternalInput")
    b3_d = nc.dram_tensor("b3", [64, 1], f32, kind="ExternalInput")
    w4_d = nc.dram_tensor("w4", [64, 17], f16, kind="ExternalInput")
    b4_d = nc.dram_tensor("b4", [17, 1], f32, kind="ExternalInput")
    w5_d = nc.dram_tensor("w5", [17, 1], f16, kind="ExternalInput")
    o_d = nc.dram_tensor("o", [1, bc], f32, kind="ExternalOutput")

    with tile.TileContext(nc) as tc, ExitStack() as ctx:
        wpool = ctx.enter_context(tc.tile_pool(name="wpool", bufs=1))
        xpool = ctx.enter_context(tc.tile_pool(name="xpool", bufs=3))
        ypool = ctx.enter_context(tc.tile_pool(name="ypool", bufs=KT + 3))
        spool = ctx.enter_context(tc.tile_pool(name="spool", bufs=2))
        ps1p = ctx.enter_context(tc.tile_pool(name="ps1p", bufs=3, space="PSUM"))
        ps2p = ctx.enter_context(tc.tile_pool(name="ps2p", bufs=1, space="PSUM"))

        # pair-0 input first so compute can start while the rest streams in
        xx_first = xpool.tile([128, PAIR_N], f16, tag="xx", name="xx_first")
        nc.sync.dma_start(xx_first[:], xx_d[:, 0:PAIR_N])
        # weight DMAs spread across the DMA-capable queues: gpsimd carries
        # the step-1-critical tensors (w1 split so the first units' weights
        # land early), scalar the later-needed step-2 weights.
        w1a_t = wpool.tile([128, 8, 128], f16, name="w1a")
        nc.gpsimd.dma_start(w1a_t[:], w1_d[:, 0:8, :])
        w1b_t = wpool.tile([128, KT - 8, 128], f16, name="w1b")
        nc.gpsimd.dma_start(w1b_t[:], w1_d[:, 8:KT, :])
        b2f_t = wpool.tile([128, 1], f32)
        nc.gpsimd.dma_start(b2f_t[:], b2f_d[:])
        w2_t = wpool.tile([128, KT, 128], f16)
        nc.scalar.dma_start(w2_t[:], w2_d[:])
        w12_t = wpool.tile([64, 128], f16)
        nc.gpsimd.dma_start(w12_t[:], w12_d[:])
        w3_t = wpool.tile([128, 64], f16)
        nc.gpsimd.dma_start(w3_t[:], w3_d[:])
        b3_t = wpool.tile([64, 1], f32)
        nc.gpsimd.dma_start(b3_t[:], b3_d[:])
        w4_t = wpool.tile([64, 17], f16)
        nc.gpsimd.dma_start(w4_t[:], w4_d[:])
        b4_t = wpool.tile([17, 1], f32)
        nc.gpsimd.dma_start(b4_t[:], b4_d[:])
        w5_t = wpool.tile([17, 1], f16)
        nc.gpsimd.dma_start(w5_t[:], w5_d[:])

        def w1s(t):
            return w1a_t[:, t, :] if t < 8 else w1b_t[:, t - 8, :]

        # PE warm-up: ~8us of dummy back-to-back K=128 matmuls on a zeroed
        # tile, bridging the startup DMA window so the HAM clock-gate is
        # open (2.4 GHz) when real work starts.
        wu_t = wpool.tile([128, 640], f16, name="wu")
        nc.vector.memset(wu_t[:], 0.0)
        wups = ps1p.tile([128, CHUNK], f32, tag="ps1", name="wups")
        for i in range(19):
            nc.tensor.matmul(wups[:], wu_t[:, 0:128], wu_t[:, 128:640],
                             start=True, stop=True)

        def make_tail_stages(p, ps2):
            """The per-pair serial tail (h -> g1 -> g2 -> out), popped at
            fixed units of the NEXT pair so the cross-engine latencies
            never head-of-line-block the in-order PE queue."""
            st = {}

            def s0():
                st["h"] = spool.tile([128, PAIR_N], f16, tag="h", name=f"h_{p}")
                nc.scalar.activation(st["h"][:], ps2[:], AF.Lrelu,
                                     bias=b2f_t[:, 0:1], alpha=LRELU_NEG)

            def s1():
                st["g1ps"] = ps1p.tile([64, PAIR_N], f32, tag="ps1",
                                       name=f"g1ps_{p}")
                for h in range(2):
                    sl = slice(h * CHUNK, (h + 1) * CHUNK)
                    nc.tensor.matmul(st["g1ps"][:, sl], w3_t[:],
                                     st["h"][:, sl], start=True, stop=True)

            def s2():
                st["g1"] = spool.tile([64, PAIR_N], f16, tag="g1",
                                      name=f"g1_{p}")
                nc.scalar.activation(st["g1"][:], st["g1ps"][:], AF.Lrelu,
                                     bias=b3_t[:, 0:1], alpha=LRELU_NEG)

            def s3():
                st["g2ps"] = ps1p.tile([17, PAIR_N], f32, tag="ps1",
                                       name=f"g2ps_{p}")
                for h in range(2):
                    sl = slice(h * CHUNK, (h + 1) * CHUNK)
                    nc.tensor.matmul(st["g2ps"][:, sl], w4_t[:],
                                     st["g1"][:, sl], start=True, stop=True)

            def s4():
                st["g2"] = spool.tile([17, PAIR_N], f16, tag="g2",
                                      name=f"g2_{p}")
                nc.scalar.activation(st["g2"][:], st["g2ps"][:], AF.Lrelu,
                                     bias=b4_t[:, 0:1], alpha=LRELU_NEG)

            def s5():
                st["ops"] = ps1p.tile([1, PAIR_N], f32, tag="ps1",
                                      name=f"ops_{p}")
                for h in range(2):
                    sl = slice(h * CHUNK, (h + 1) * CHUNK)
                    nc.tensor.matmul(st["ops"][:, sl], w5_t[:],
                                     st["g2"][:, sl], start=True, stop=True)

            def s6():
                o_t = spool.tile([1, PAIR_N], f32, tag="o", name=f"o_{p}")
                nc.vector.tensor_copy(o_t[:], st["ops"][:])
                nc.sync.dma_start(o_d[:, p * PAIR_N:(p + 1) * PAIR_N], o_t[:])

            return [s0, s1, s2, s3, s4, s5, s6]

        tail_stages = []
        # tail stage -> unit of the NEXT pair it pops at.  Units 0-3 are
        # reserved for the previous pair's carried chain steps, so the
        # h ACT (stage 0) pops at unit 4 and the rest follow.
        TAIL_UNIT = {0: 4, 1: 7, 2: 9, 3: 11, 4: 13, 5: 15, 6: 17}

        def make_chain_emitter(ps2, ytiles, chain_pos, chain_len):
            def emit(lag_tile, max_rounds):
                """Emit up to max_rounds chunk-major rounds of ready chain
                steps: all ready groups' h0 chunks back-to-back, then their
                h1 chunks, so up to four 32-column tile positions stream
                concurrently."""
                for _ in range(max_rounds):
                    rg = [g for g in range(4)
                          if chain_pos[g] < chain_len[g]
                          and GROUP_TR[g][0] + chain_pos[g] <= lag_tile]
                    if not rg:
                        return
                    for h in range(2):
                        sl = slice(h * CHUNK, (h + 1) * CHUNK)
                        for g in rg:
                            t = GROUP_TR[g][0] + chain_pos[g]
                            nc.tensor.matmul(
                                ps2[32 * g:32 * (g + 1), sl],
                                w2_t[:, t, 32 * g:32 * (g + 1)],
                                ytiles[t][:, sl], start=False,
                                stop=(chain_pos[g] == chain_len[g] - 1),
                                tile_position=(0, 32 * g),
                                skip_group_check=True)
                    for g in rg:
                        chain_pos[g] += 1

            def remaining():
                return any(chain_pos[g] < chain_len[g] for g in range(4))

            return emit, remaining

        carry_emit = None        # leftover chain steps of the previous pair
        carry_rem = None

        xx_next = xx_first
        for p in range(n_pairs):
            xx_t = xx_next
            stages = tail_stages
            tail_stages = []

            ps2 = ps2p.tile([128, PAIR_N], f32, tag="ps2", name=f"ps2_{p}")
            ytiles = [None] * KT
            chain_len = [t1 - t0 for t0, t1 in GROUP_TR]
            chain_pos = [0] * 4
            emit_chain_rounds, chain_remaining = make_chain_emitter(
                ps2, ytiles, chain_pos, chain_len)

            stage_i = 0
            for u in range(KT):
                while (stage_i < len(stages)
                       and TAIL_UNIT[stage_i] <= u):
                    stages[stage_i]()
                    stage_i += 1
                if u <= 3 and carry_emit is not None:
                    # previous pair's carried chain steps, interleaved into
                    # this pair's first units so the PE queue never parks a
                    # whole flush (which would starve the evac engines at
                    # the boundary); force-drained by the end of unit 3
                    # because the h ACT of that pair pops at unit 4.
                    carry_emit(KT, 2 if u < 3 else 12)
                    if u == 3:
                        assert not carry_rem()
                        carry_emit = carry_rem = None
                if p + 1 < n_pairs and u == 2:
                    xx_next = xpool.tile([128, PAIR_N], f16, tag="xx",
                                         name=f"xx_{p + 1}")
                    nc.sync.dma_start(
                        xx_next[:],
                        xx_d[:, (p + 1) * PAIR_N:(p + 2) * PAIR_N])

                psU = ps1p.tile([128, PAIR_N], f32, tag="ps1",
                                name=f"psU_{p}_{u}")
                for h in range(2):
                    sl = slice(h * CHUNK, (h + 1) * CHUNK)
                    nc.tensor.matmul(psU[:, sl], w1s(u), xx_t[:, sl],
                                     start=True, stop=True)
                y_t = ypool.tile([128, PAIR_N], f16, tag="y",
                                 name=f"y_{p}_{u}")
                if u in VEC_TILES:
                    nc.vector.tensor_scalar_max(y_t[:], psU[:], 0.0)
                else:
                    nc.scalar.activation(y_t[:], psU[:], AF.Lrelu,
                                         alpha=LRELU_NEG)
                ytiles[u] = y_t

                if u == 7:
                    # first write of ps2: the 0.01*y fold path (vector tiles
                    # only).  Emitted late enough that the previous pair's
                    # h-act (popped at unit 4) has released the bank.
                    for h in range(2):
                        sl = slice(h * CHUNK, (h + 1) * CHUNK)
                        nc.tensor.matmul(
                            ps2[:, sl], w12_t[:], xx_t[0:64, sl],
                            start=True, stop=False, tile_position=(0, 0),
                            skip_group_check=True)
                if u >= 8:
                    emit_chain_rounds(u - 2, 1)

            # ---- the rest of this pair's chain steps are carried into the
            # next pair's units 0-3 (or flushed below for the last pair).
            while stage_i < len(stages):
                stages[stage_i]()
                stage_i += 1
            carry_emit, carry_rem = emit_chain_rounds, chain_remaining
            tail_stages = make_tail_stages(p, ps2)

        while carry_rem():
            carry_emit(KT, 1)
        for st in tail_stages:
            st()

    nc.compile()
    return nc


# ---------------------------------------------------------------- execution
_NC_CACHE = {}
LAST_RESULT = None


def _prep_inputs(inputs):
    board = np.ascontiguousarray(np.asarray(inputs["board"], np.float32))
    x = board.reshape(B_TOTAL, 64)
    dev = _fold_params(inputs)
    ones = np.ones((1, BC), np.float16)
    zeros = np.zeros((63, BC), np.float16)
    in_maps = []
    for c in range(N_CORES):
        xc = np.ascontiguousarray(x[c * BC:(c + 1) * BC].T)      # [64, BC]
        m = dict(dev)
        m["xx"] = np.ascontiguousarray(
            np.vstack([xc.astype(np.float16), ones, zeros]))     # [128, BC]
        in_maps.append(m)
    return in_maps


def kernel(**inputs):
    global LAST_RESULT
    from concourse.bass_utils import run_bass_kernel_spmd

    if "nc" not in _NC_CACHE:
        _NC_CACHE["nc"] = _build_nc()
    nc = _NC_CACHE["nc"]

    in_maps = _prep_inputs(inputs)
    res = run_bass_kernel_spmd(nc, in_maps, core_ids=list(range(N_CORES)))
    LAST_RESULT = res
    out = np.concatenate([r["o"].reshape(-1) for r in res.results])
    return out.reshape(B_TOTAL, 1).astype(np.float32)


# revision 17
# speedup vs baseline: 1.0881x; 1.0881x over previous
"""Trainium2 Bass kernel for nn_BlockBlastValueNet1PmultikernelFlattenned.

Strategy (v10)
--------------
The network is 8 tiny conv branches over an 8x8 board followed by small MLPs.
Because the board has only 64 pixels, every conv branch (pad const 1.0 +
valid conv + bias) is an affine map of the 64 board values.  The whole net
folds into:

    y  = x @ W1 + c1                     # [B, NF]  (NF = 2944 padded)
    h  = Lrelu( Lrelu(y) @ W2' + b2 )    # per-branch first FC, block diagonal
    g1 = Lrelu( h @ W3 + b3 )            # branch second FC fused with fc1
    g2 = Lrelu( g1 @ W4 + b4 )           # fc2 (augmented with a ones column)
    out = g2 @ W5                        # fc3 (bias folded via augmentation)

Data-parallel over 8 NeuronCores (batch 65536 -> 8192/core), processed in
8 pair-iterations of 1024 samples.  Features live on SBUF partitions,
samples stream on the free dim.  All matmuls are fp16, 512-wide chunks.

Two hardware constraints shape the schedule:

1. PSUM evacuation is the floor: every y element must pass through the
   Scalar or Vector engine exactly once (~1 elem/cycle/lane each), so both
   engines must stay ~100% busy.  y tiles are split ~11/12 between Scalar
   (exact Lrelu ACT) and Vector (relu via tensor_scalar max; the 0.01*y
   leaky path of vector tiles folds into a small 64->128 matmul W12).
2. The PE HAM clock-gate measures *array activity*, not just busyness:
   half-array (K=64) matmuls count ~50% active and the PE gets re-throttled
   to 1.2 GHz during any K=64-dominated phase, doubling matmul time.  So
   step-1 runs as full K=128 matmuls: the input is [x; 1; zeros] and each
   step-1 weight tile carries the y-bias c1 in row 64 — same matmul cost
   (cost depends only on moving width), full activity, and the bias lands
   in PSUM for free so evacuations need no per-tile bias operand.

Schedule per 1024-sample pair: 23 "units" (one y tile each: two 512-wide
step-1 matmuls + one evacuation), with the step-2 column chains (four
32-column accumulation chains at tile_position (0,32g), one per branch
pair-group, K=128) emitted as chunk-major rounds between units, lagging
evacuation by ~2 units.  Chain steps that would wait on the last
evacuations are carried into the NEXT pair's units 0-3 so the in-order PE
queue never parks a flush while the evac engines starve.  The serial tail
(h -> g1 -> g2 -> out) pops at fixed units of the next pair.  A ~8us
warm-up block of dummy K=128 matmuls bridges the startup DMA window so
the HAM gate is open before real work starts.
"""

import numpy as np

# ---------------------------------------------------------------- constants
SPECS = [(1, 1, 1, 0, 0), (2, 2, 6, 1, 1), (3, 3, 8, 1, 1), (4, 4, 8, 2, 2),
         (5, 5, 16, 2, 2), (8, 8, 32, 0, 0), (1, 8, 4, 0, 0), (8, 1, 4, 0, 0)]
BOARD = 8
B_TOTAL = 65536
N_CORES = 8
BC = B_TOTAL // N_CORES          # 8192 samples per core
PAIR_N = 1024                    # samples per pair-iteration
CHUNK = 512                      # matmul moving width (1 psum bank fp32)
N_PAIRS = BC // PAIR_N           # 8

# column-group pairing of branches (2 branches x 16 h-outputs = 32 cols each).
# y rows are packed tightly in group order; a K-tile straddling a group
# boundary appears in BOTH groups' chains (the block-diagonal W2 already has
# zeros for the other group's columns).
GROUPS = [[4, 5], [3, 6], [2, 7], [1, 0]]
_BR_N = []
for kh, kw, fs, ph, pw in SPECS:
    _BR_N.append((BOARD + 2 * ph - kh + 1) * (BOARD + 2 * pw - kw + 1) * fs)
_NF_TRUE = sum(_BR_N)            # 2830
KT = -(-_NF_TRUE // 128)         # 23 K-tiles / M-tiles of y
NF = KT * 128                    # 2944 (last tile zero-padded)
_G_ROWS = []
_o = 0
for g in GROUPS:
    sz = sum(_BR_N[b] for b in g)
    _G_ROWS.append((_o, _o + sz))
    _o += sz
GROUP_TR = [(r0 // 128, -(-r1 // 128)) for r0, r1 in _G_ROWS]  # tile ranges

LRELU_NEG = 0.01

# Unit order: y tiles are processed interleaved across the four chain
# groups (round r of the chains needs tiles [r, 8+r, 13+r, 17+r]), so all
# four column chains become ready after four units and the chain rounds
# pack 4-way from the start of the pair instead of only near its end.
U_SEQ = [0, 8, 13, 17, 1, 9, 14, 18, 2, 10, 15, 19,
         3, 11, 16, 20, 4, 12, 21, 5, 22, 6, 7]
assert sorted(U_SEQ) == list(range(KT))

# evac engine split by unit position: the first 4 units go to the Vector
# engine (at pair start the Scalar engine still runs the previous pair's
# h activation), then alternate.  VEC_TILES is the induced tile set (used
# by the host fold for the relu-decomposition bookkeeping).
_VEC_POS = frozenset({0, 1, 2, 3, 5, 7, 9, 11, 13, 15, 17, 19})
VEC_TILES = frozenset(U_SEQ[i] for i in _VEC_POS)


# ---------------------------------------------------------------- host fold
def _fold_params(p):
    """Fold conv branches + MLPs into the dense pipeline weights (float64)."""
    n_of = _BR_N
    W1_of, c1_of = {}, {}
    for i, (kh, kw, fs, ph, pw) in enumerate(SPECS):
        Ho = BOARD + 2 * ph - kh + 1
        Wo = BOARD + 2 * pw - kw + 1
        cw = np.asarray(p[f"b{i}_cw"], np.float64)
        cb = np.asarray(p[f"b{i}_cb"], np.float64)
        W1 = np.zeros((64, n_of[i]))
        c1 = np.zeros((n_of[i],))
        for f in range(fs):
            for oh in range(Ho):
                for ow in range(Wo):
                    oi = (f * Ho + oh) * Wo + ow
                    c1[oi] += cb[f]
                    for u in range(kh):
                        for v in range(kw):
                            r, c = oh + u - ph, ow + v - pw
                            w = cw[f, 0, u, v]
                            if 0 <= r < 8 and 0 <= c < 8:
                                W1[r * 8 + c, oi] += w
                            else:
                                c1[oi] += w        # pad value is 1.0
        W1_of[i] = W1
        c1_of[i] = c1

    # K-layout: groups concatenated tightly; only the final tile is padded
    K_start = {}
    off = 0
    for g in GROUPS:
        for b in g:
            K_start[b] = off
            off += n_of[b]
    assert off == _NF_TRUE
    border = [b for g in GROUPS for b in g]       # h block order
    hpos = {b: j * 16 for j, b in enumerate(border)}

    W1p = np.zeros((64, NF))
    c1p = np.zeros((NF,))
    W2p = np.zeros((NF, 128))
    b2p = np.zeros((128,))
    for b in range(8):
        s, n, hp = K_start[b], n_of[b], hpos[b]
        W1p[:, s:s + n] = W1_of[b]
        c1p[s:s + n] = c1_of[b]
        W2p[s:s + n, hp:hp + 16] = np.asarray(p[f"b{b}_w1"], np.float64).T
        b2p[hp:hp + 16] = np.asarray(p[f"b{b}_b1"], np.float64)

    Wb = np.zeros((128, 64))
    bb = np.zeros((64,))
    for b in range(8):
        hp = hpos[b]
        Wb[hp:hp + 16, 8 * b:8 * b + 8] = np.asarray(p[f"b{b}_w2"], np.float64).T
        bb[8 * b:8 * b + 8] = np.asarray(p[f"b{b}_b2"], np.float64)
    fc_w1 = np.asarray(p["fc_w1"], np.float64)
    fc_b1 = np.asarray(p["fc_b1"], np.float64)
    W3 = Wb @ fc_w1.T
    b3 = bb @ fc_w1.T + fc_b1
    fc_w2 = np.asarray(p["fc_w2"], np.float64)
    fc_b2 = np.asarray(p["fc_b2"], np.float64)
    fc_w3 = np.asarray(p["fc_w3"], np.float64)
    fc_b3 = np.asarray(p["fc_b3"], np.float64)
    W4 = np.zeros((64, 17)); W4[:, :16] = fc_w2.T
    b4 = np.zeros((17,)); b4[:16] = fc_b2; b4[16] = 1.0
    W5 = np.zeros((17, 1)); W5[:16, 0] = fc_w3[0]; W5[16, 0] = fc_b3[0]

    # Vector-assigned tiles use relu + decomposition:
    #   Lrelu(v) = 0.01*v + 0.99*relu(v)
    # so their W2 rows are scaled by 0.99 and the 0.01*v path (affine in x)
    # folds into W12 / b2f.  Scalar-assigned tiles compute Lrelu exactly on
    # the ACT engine, so their W2 rows stay unscaled and contribute nothing
    # to the fold.
    vmask = np.zeros((NF, 1))
    for t in range(KT):
        if t in VEC_TILES:
            vmask[128 * t:128 * (t + 1)] = 1.0
    W2s = np.where(vmask > 0, (1.0 - LRELU_NEG) * W2p, W2p)
    W12 = LRELU_NEG * ((W1p * vmask[:, 0][None, :]) @ W2p)
    b2f = LRELU_NEG * ((c1p * vmask[:, 0]) @ W2p) + b2p

    f32 = np.float32
    f16 = np.float16
    dev = {}
    # step-1 weight tiles: full K=128 (input is [x; 1; zeros]); row 64
    # carries the y-bias c1 so the PSUM result is already biased.  Packed
    # in unit order so the first DMA chunk covers the first units.
    w1 = np.zeros((128, KT, 128), f16)
    for i, t in enumerate(U_SEQ):
        w1[0:64, i, :] = W1p[:, 128 * t:128 * (t + 1)]
        w1[64, i, :] = c1p[128 * t:128 * (t + 1)]
    dev["w1"] = w1
    w2 = np.zeros((128, KT, 128), f16)
    for t in range(KT):
        w2[:, t, :] = W2s[128 * t:128 * (t + 1), :]
    dev["w2"] = w2
    dev["w12"] = W12.astype(f16)
    dev["b2f"] = b2f.reshape(128, 1).astype(f32)
    dev["w3"] = W3.astype(f16)
    dev["b3"] = b3.reshape(64, 1).astype(f32)
    dev["w4"] = W4.astype(f16)
    dev["b4"] = b4.reshape(17, 1).astype(f32)
    dev["w5"] = W5.astype(f16)
    return dev


# ---------------------------------------------------------------- device IR
def _build_nc(n_pairs=N_PAIRS):
    import concourse.mybir as mybir
    import concourse.tile as tile
    from concourse import bacc
    from contextlib import ExitStack

    dt = mybir.dt
    AF = mybir.ActivationFunctionType
    f32 = dt.float32
    f16 = dt.float16
    bc = n_pairs * PAIR_N

    nc = bacc.Bacc("TRN2", target_bir_lowering=False, debug=False,
                   num_devices=N_CORES)

    xx_d = nc.dram_tensor("xx", [128, bc], f16, kind="ExternalInput")
    w1_d = nc.dram_tensor("w1", [128, KT, 128], f16, kind="ExternalInput")
    w2_d = nc.dram_tensor("w2", [128, KT, 128], f16, kind="ExternalInput")
    w12_d = nc.dram_tensor("w12", [64, 128], f16, kind="ExternalInput")
    b2f_d = nc.dram_tensor("b2f", [128, 1], f32, kind="ExternalInput")
    w3_d = nc.dram_tensor("w3", [128, 64], f16, kind="ExternalInput")
    b3_d = nc.dram_tensor("b3", [64, 1], f32, kind="ExternalInput")
    w4_d = nc.dram_tensor("w4", [64, 17], f16, kind="ExternalInput")
    b4_d = nc.dram_tensor("b4", [17, 1], f32, kind="ExternalInput")
    w5_d = nc.dram_tensor("w5", [17, 1], f16, kind="ExternalInput")
    o_d = nc.dram_tensor("o", [1, bc], f32, kind="ExternalOutput")

    with tile.TileContext(nc) as tc, ExitStack() as ctx:
        wpool = ctx.enter_context(tc.tile_pool(name="wpool", bufs=1))
        xpool = ctx.enter_context(tc.tile_pool(name="xpool", bufs=3))
        ypool = ctx.enter_context(tc.tile_pool(name="ypool", bufs=KT + 3))
        spool = ctx.enter_context(tc.tile_pool(name="spool", bufs=2))
        ps1p = ctx.enter_context(tc.tile_pool(name="ps1p", bufs=3, space="PSUM"))
        ps2p = ctx.enter_context(tc.tile_pool(name="ps2p", bufs=1, space="PSUM"))

        # pair-0 input first so compute can start while the rest streams in
        xx_first = xpool.tile([128, PAIR_N], f16, tag="xx", name="xx_first")
        nc.sync.dma_start(xx_first[:], xx_d[:, 0:PAIR_N])
        # weight DMAs spread across the DMA-capable queues: gpsimd carries
        # the step-1-critical tensors (w1 split so the first units' weights
        # land early), scalar the later-needed step-2 weights.
        w1a_t = wpool.tile([128, 8, 128], f16, name="w1a")
        nc.gpsimd.dma_start(w1a_t[:], w1_d[:, 0:8, :])
        w1b_t = wpool.tile([128, KT - 8, 128], f16, name="w1b")
        nc.gpsimd.dma_start(w1b_t[:], w1_d[:, 8:KT, :])
        b2f_t = wpool.tile([128, 1], f32)
        nc.gpsimd.dma_start(b2f_t[:], b2f_d[:])
        w2_t = wpool.tile([128, KT, 128], f16)
        nc.scalar.dma_start(w2_t[:], w2_d[:])
        w12_t = wpool.tile([64, 128], f16)
        nc.gpsimd.dma_start(w12_t[:], w12_d[:])
        w3_t = wpool.tile([128, 64], f16)
        nc.gpsimd.dma_start(w3_t[:], w3_d[:])
        b3_t = wpool.tile([64, 1], f32)
        nc.gpsimd.dma_start(b3_t[:], b3_d[:])
        w4_t = wpool.tile([64, 17], f16)
        nc.gpsimd.dma_start(w4_t[:], w4_d[:])
        b4_t = wpool.tile([17, 1], f32)
        nc.gpsimd.dma_start(b4_t[:], b4_d[:])
        w5_t = wpool.tile([17, 1], f16)
        nc.gpsimd.dma_start(w5_t[:], w5_d[:])

        def w1s(t):
            return w1a_t[:, t, :] if t < 8 else w1b_t[:, t - 8, :]

        # PE warm-up: ~8us of dummy back-to-back K=128 matmuls on a zeroed
        # tile, bridging the startup DMA window so the HAM clock-gate is
        # open (2.4 GHz) when real work starts.
        wu_t = wpool.tile([128, 640], f16, name="wu")
        nc.vector.memset(wu_t[:], 0.0)
        wups = ps1p.tile([128, CHUNK], f32, tag="ps1", name="wups")
        for i in range(19):
            nc.tensor.matmul(wups[:], wu_t[:, 0:128], wu_t[:, 128:640],
                             start=True, stop=True)

        def make_tail_stages(p, ps2):
            """The per-pair serial tail (h -> g1 -> g2 -> out), popped at
            fixed units of the NEXT pair so the cross-engine latencies
            never head-of-line-block the in-order PE queue."""
            st = {}

            def s0():
                st["h"] = spool.tile([128, PAIR_N], f16, tag="h", name=f"h_{p}")
                nc.scalar.activation(st["h"][:], ps2[:], AF.Lrelu,
                                     bias=b2f_t[:, 0:1], alpha=LRELU_NEG)

            def s1():
                st["g1ps"] = ps1p.tile([64, PAIR_N], f32, tag="ps1",
                                       name=f"g1ps_{p}")
                for h in range(2):
                    sl = slice(h * CHUNK, (h + 1) * CHUNK)
                    nc.tensor.matmul(st["g1ps"][:, sl], w3_t[:],
                                     st["h"][:, sl], start=True, stop=True)

            def s2():
                st["g1"] = spool.tile([64, PAIR_N], f16, tag="g1",
                                      name=f"g1_{p}")
                nc.scalar.activation(st["g1"][:], st["g1ps"][:], AF.Lrelu,
                                     bias=b3_t[:, 0:1], alpha=LRELU_NEG)

            def s3():
                st["g2ps"] = ps1p.tile([17, PAIR_N], f32, tag="ps1",
                                       name=f"g2ps_{p}")
                for h in range(2):
                    sl = slice(h * CHUNK, (h + 1) * CHUNK)
                    nc.tensor.matmul(st["g2ps"][:, sl], w4_t[:],
                                     st["g1"][:, sl], start=True, stop=True)

            def s4():
                st["g2"] = spool.tile([17, PAIR_N], f16, tag="g2",
                                      name=f"g2_{p}")
                nc.scalar.activation(st["g2"][:], st["g2ps"][:], AF.Lrelu,
                                     bias=b4_t[:, 0:1], alpha=LRELU_NEG)

            def s5():
                st["ops"] = ps1p.tile([1, PAIR_N], f32, tag="ps1",
                                      name=f"ops_{p}")
                for h in range(2):
                    sl = slice(h * CHUNK, (h + 1) * CHUNK)
                    nc.tensor.matmul(st["ops"][:, sl], w5_t[:],
                                     st["g2"][:, sl], start=True, stop=True)

            def s6():
                o_t = spool.tile([1, PAIR_N], f32, tag="o", name=f"o_{p}")
                nc.vector.tensor_copy(o_t[:], st["ops"][:])
                nc.sync.dma_start(o_d[:, p * PAIR_N:(p + 1) * PAIR_N], o_t[:])

            return [s0, s1, s2, s3, s4, s5, s6]

        tail_stages = []
        # tail stage -> unit of the NEXT pair it pops at.  Units 0-3 are
        # reserved for the previous pair's carried chain steps, so the
        # h ACT (stage 0) pops at unit 4 and the rest follow.
        TAIL_UNIT = {0: 4, 1: 8, 2: 10, 3: 12, 4: 14, 5: 16, 6: 18}

        def make_chain_emitter(ps2, ytiles, chain_pos, chain_len, emit_unit):
            def emit(now_unit, max_rounds):
                """Emit up to max_rounds chunk-major rounds of ready chain
                steps (a step is ready once its y tile's evacuation was
                emitted >= 3 units ago): all ready groups' h0 chunks
                back-to-back, then their h1 chunks, so up to four 32-column
                tile positions stream concurrently."""
                for _ in range(max_rounds):
                    rg = [g for g in range(4)
                          if chain_pos[g] < chain_len[g]
                          and emit_unit.get(GROUP_TR[g][0] + chain_pos[g],
                                            99) <= now_unit - 3]
                    if not rg:
                        return
                    for h in range(2):
                        sl = slice(h * CHUNK, (h + 1) * CHUNK)
                        for g in rg:
                            t = GROUP_TR[g][0] + chain_pos[g]
                            nc.tensor.matmul(
                                ps2[32 * g:32 * (g + 1), sl],
                                w2_t[:, t, 32 * g:32 * (g + 1)],
                                ytiles[t][:, sl], start=False,
                                stop=(chain_pos[g] == chain_len[g] - 1),
                                tile_position=(0, 32 * g),
                                skip_group_check=True)
                    for g in rg:
                        chain_pos[g] += 1

            def remaining():
                return any(chain_pos[g] < chain_len[g] for g in range(4))

            return emit, remaining

        carry_emit = None        # leftover chain steps of the previous pair
        carry_rem = None

        xx_next = xx_first
        for p in range(n_pairs):
            xx_t = xx_next
            stages = tail_stages
            tail_stages = []

            ps2 = ps2p.tile([128, PAIR_N], f32, tag="ps2", name=f"ps2_{p}")
            ytiles = [None] * KT
            emit_unit = {}
            chain_len = [t1 - t0 for t0, t1 in GROUP_TR]
            chain_pos = [0] * 4
            emit_chain_rounds, chain_remaining = make_chain_emitter(
                ps2, ytiles, chain_pos, chain_len, emit_unit)

            stage_i = 0
            for u, t in enumerate(U_SEQ):
                while (stage_i < len(stages)
                       and TAIL_UNIT[stage_i] <= u):
                    stages[stage_i]()
                    stage_i += 1
                if u <= 3 and carry_emit is not None:
                    # previous pair's carried chain steps, interleaved into
                    # this pair's first units so the PE queue never parks a
                    # whole flush (which would starve the evac engines at
                    # the boundary); force-drained by the end of unit 3
                    # because the h ACT of that pair pops at unit 4.
                    carry_emit(99, 2 if u < 3 else 12)
                    if u == 3:
                        assert not carry_rem()
                        carry_emit = carry_rem = None
                if p + 1 < n_pairs and u == 2:
                    xx_next = xpool.tile([128, PAIR_N], f16, tag="xx",
                                         name=f"xx_{p + 1}")
                    nc.sync.dma_start(
                        xx_next[:],
                        xx_d[:, (p + 1) * PAIR_N:(p + 2) * PAIR_N])

                psU = ps1p.tile([128, PAIR_N], f32, tag="ps1",
                                name=f"psU_{p}_{u}")
                for h in range(2):
                    sl = slice(h * CHUNK, (h + 1) * CHUNK)
                    nc.tensor.matmul(psU[:, sl], w1s(u), xx_t[:, sl],
                                     start=True, stop=True)
                y_t = ypool.tile([128, PAIR_N], f16, tag="y",
                                 name=f"y_{p}_{t}")
                if u in _VEC_POS:
                    nc.vector.tensor_scalar_max(y_t[:], psU[:], 0.0)
                else:
                    nc.scalar.activation(y_t[:], psU[:], AF.Lrelu,
                                         alpha=LRELU_NEG)
                ytiles[t] = y_t
                emit_unit[t] = u

                if u == 6:
                    # first write of ps2: the 0.01*y fold path (vector tiles
                    # only).  Emitted late enough that the previous pair's
                    # h-act (popped at unit 4) has released the bank.
                    for h in range(2):
                        sl = slice(h * CHUNK, (h + 1) * CHUNK)
                        nc.tensor.matmul(
                            ps2[:, sl], w12_t[:], xx_t[0:64, sl],
                            start=True, stop=False, tile_position=(0, 0),
                            skip_group_check=True)
                if u in (7, 10, 14, 18, 22):
                    # batched chain points: fewer tile-position switches on
                    # the PE (each switch costs ~106ns of un-hidden
                    # LDWEIGHTS), and each point's rounds are 4-way packed
                    # thanks to the interleaved unit order.
                    emit_chain_rounds(u, 2)

            # ---- the rest of this pair's chain steps are carried into the
            # next pair's units 0-3 (or flushed below for the last pair).
            while stage_i < len(stages):
                stages[stage_i]()
                stage_i += 1
            carry_emit, carry_rem = emit_chain_rounds, chain_remaining
            tail_stages = make_tail_stages(p, ps2)

        while carry_rem():
            carry_emit(99, 1)
        for st in tail_stages:
            st()

    nc.compile()
    return nc


# ---------------------------------------------------------------- execution
_NC_CACHE = {}
LAST_RESULT = None


def _prep_inputs(inputs):
    board = np.ascontiguousarray(np.asarray(inputs["board"], np.float32))
    x = board.reshape(B_TOTAL, 64)
    dev = _fold_params(inputs)
    ones = np.ones((1, BC), np.float16)
    zeros = np.zeros((63, BC), np.float16)
    in_maps = []
    for c in range(N_CORES):
        xc = np.ascontiguousarray(x[c * BC:(c + 1) * BC].T)      # [64, BC]
        m = dict(dev)
        m["xx"] = np.ascontiguousarray(
            np.vstack([xc.astype(np.float16), ones, zeros]))     # [128, BC]
        in_maps.append(m)
    return in_maps


def kernel(**inputs):
    global LAST_RESULT
    from concourse.bass_utils import run_bass_kernel_spmd

    if "nc" not in _NC_CACHE:
        _NC_CACHE["nc"] = _build_nc()
    nc = _NC_CACHE["nc"]

    in_maps = _prep_inputs(inputs)
    res = run_bass_kernel_spmd(nc, in_maps, core_ids=list(range(N_CORES)))
    LAST_RESULT = res
    out = np.concatenate([r["o"].reshape(-1) for r in res.results])
    return out.reshape(B_TOTAL, 1).astype(np.float32)


# revision 20
# speedup vs baseline: 1.1037x; 1.0143x over previous
"""Trainium2 Bass kernel for nn_BlockBlastValueNet1PmultikernelFlattenned.

Strategy (v10)
--------------
The network is 8 tiny conv branches over an 8x8 board followed by small MLPs.
Because the board has only 64 pixels, every conv branch (pad const 1.0 +
valid conv + bias) is an affine map of the 64 board values.  The whole net
folds into:

    y  = x @ W1 + c1                     # [B, NF]  (NF = 2944 padded)
    h  = Lrelu( Lrelu(y) @ W2' + b2 )    # per-branch first FC, block diagonal
    g1 = Lrelu( h @ W3 + b3 )            # branch second FC fused with fc1
    g2 = Lrelu( g1 @ W4 + b4 )           # fc2 (augmented with a ones column)
    out = g2 @ W5                        # fc3 (bias folded via augmentation)

Data-parallel over 8 NeuronCores (batch 65536 -> 8192/core), processed in
8 pair-iterations of 1024 samples.  Features live on SBUF partitions,
samples stream on the free dim.  All matmuls are fp16, 512-wide chunks.

Two hardware constraints shape the schedule:

1. PSUM evacuation is the floor: every y element must pass through the
   Scalar or Vector engine exactly once (~1 elem/cycle/lane each), so both
   engines must stay ~100% busy.  y tiles are split ~11/12 between Scalar
   (exact Lrelu ACT) and Vector (relu via tensor_scalar max; the 0.01*y
   leaky path of vector tiles folds into a small 64->128 matmul W12).
2. The PE HAM clock-gate measures *array activity*, not just busyness:
   half-array (K=64) matmuls count ~50% active and the PE gets re-throttled
   to 1.2 GHz during any K=64-dominated phase, doubling matmul time.  So
   step-1 runs as full K=128 matmuls: the input is [x; 1; zeros] and each
   step-1 weight tile carries the y-bias c1 in row 64 — same matmul cost
   (cost depends only on moving width), full activity, and the bias lands
   in PSUM for free so evacuations need no per-tile bias operand.

Schedule per 1024-sample pair: 23 "units" (one y tile each: two 512-wide
step-1 matmuls + one evacuation), with the step-2 column chains (four
32-column accumulation chains at tile_position (0,32g), one per branch
pair-group, K=128) emitted as chunk-major rounds between units, lagging
evacuation by ~2 units.  Chain steps that would wait on the last
evacuations are carried into the NEXT pair's units 0-3 so the in-order PE
queue never parks a flush while the evac engines starve.  The serial tail
(h -> g1 -> g2 -> out) pops at fixed units of the next pair.  A ~8us
warm-up block of dummy K=128 matmuls bridges the startup DMA window so
the HAM gate is open before real work starts.
"""

import numpy as np

# ---------------------------------------------------------------- constants
SPECS = [(1, 1, 1, 0, 0), (2, 2, 6, 1, 1), (3, 3, 8, 1, 1), (4, 4, 8, 2, 2),
         (5, 5, 16, 2, 2), (8, 8, 32, 0, 0), (1, 8, 4, 0, 0), (8, 1, 4, 0, 0)]
BOARD = 8
B_TOTAL = 65536
N_CORES = 8
BC = B_TOTAL // N_CORES          # 8192 samples per core
PAIR_N = 1024                    # samples per pair-iteration
CHUNK = 512                      # matmul moving width (1 psum bank fp32)
N_PAIRS = BC // PAIR_N           # 8

# column-group pairing of branches (2 branches x 16 h-outputs = 32 cols each).
# y rows are packed tightly in group order; a K-tile straddling a group
# boundary appears in BOTH groups' chains (the block-diagonal W2 already has
# zeros for the other group's columns).
GROUPS = [[4, 5], [3, 6], [2, 7], [1, 0]]
_BR_N = []
for kh, kw, fs, ph, pw in SPECS:
    _BR_N.append((BOARD + 2 * ph - kh + 1) * (BOARD + 2 * pw - kw + 1) * fs)
_NF_TRUE = sum(_BR_N)            # 2830
KT = -(-_NF_TRUE // 128)         # 23 K-tiles / M-tiles of y
NF = KT * 128                    # 2944 (last tile zero-padded)
_G_ROWS = []
_o = 0
for g in GROUPS:
    sz = sum(_BR_N[b] for b in g)
    _G_ROWS.append((_o, _o + sz))
    _o += sz
GROUP_TR = [(r0 // 128, -(-r1 // 128)) for r0, r1 in _G_ROWS]  # tile ranges

LRELU_NEG = 0.01

# Step-2 chain round plan.  The four column chains run g0 forward (tiles
# 0..8), g1 backward (13..8), g2 forward (13..17), g3 backward (22..17):
# accumulation order is free, and this way each tile shared between two
# neighbouring groups is reached by both chains in the same round and can
# be emitted as a single 64-column matmul (saving 6 matmuls/pair), with a
# consistent stop flag (every PSUM column is closed exactly once).
# Each round entry is (col_offset, col_width, tile, stop).
ROUND_PLAN = [
    [(0, 32, 0, False), (32, 32, 13, False), (64, 32, 13, False),
     (96, 32, 22, False)],
    [(0, 32, 1, False), (32, 32, 12, False), (64, 32, 14, False),
     (96, 32, 21, False)],
    [(0, 32, 2, False), (32, 32, 11, False), (64, 32, 15, False),
     (96, 32, 20, False)],
    [(0, 32, 3, False), (32, 32, 10, False), (64, 32, 16, False),
     (96, 32, 19, False)],
    [(0, 32, 4, False), (32, 32, 9, False), (96, 32, 18, False)],
    [(0, 32, 5, False), (64, 64, 17, True)],
    [(0, 32, 6, False)],
    [(0, 32, 7, False)],
    [(0, 64, 8, True)],
]
assert sorted({t for r in ROUND_PLAN for (_, _, t, _) in r}) == list(range(KT))

# Unit order: y tiles are processed in chain-round order so every round's
# tiles are evacuated a few units before the round is emitted and the
# rounds pack up to 4 concurrent tile positions from the start of the pair.
U_SEQ = []
for _r in ROUND_PLAN:
    for (_, _, _t, _) in _r:
        if _t not in U_SEQ:
            U_SEQ.append(_t)
assert sorted(U_SEQ) == list(range(KT))

# evac engine split by unit position: the first 4 units go to the Vector
# engine (at pair start the Scalar engine still runs the previous pair's
# h activation), then alternate.  VEC_TILES is the induced tile set (used
# by the host fold for the relu-decomposition bookkeeping).
_VEC_POS = frozenset({0, 1, 2, 3, 5, 7, 9, 11, 13, 15, 17, 19})
VEC_TILES = frozenset(U_SEQ[i] for i in _VEC_POS)


# ---------------------------------------------------------------- host fold
def _fold_params(p):
    """Fold conv branches + MLPs into the dense pipeline weights (float64)."""
    n_of = _BR_N
    W1_of, c1_of = {}, {}
    for i, (kh, kw, fs, ph, pw) in enumerate(SPECS):
        Ho = BOARD + 2 * ph - kh + 1
        Wo = BOARD + 2 * pw - kw + 1
        cw = np.asarray(p[f"b{i}_cw"], np.float64)
        cb = np.asarray(p[f"b{i}_cb"], np.float64)
        W1 = np.zeros((64, n_of[i]))
        c1 = np.zeros((n_of[i],))
        for f in range(fs):
            for oh in range(Ho):
                for ow in range(Wo):
                    oi = (f * Ho + oh) * Wo + ow
                    c1[oi] += cb[f]
                    for u in range(kh):
                        for v in range(kw):
                            r, c = oh + u - ph, ow + v - pw
                            w = cw[f, 0, u, v]
                            if 0 <= r < 8 and 0 <= c < 8:
                                W1[r * 8 + c, oi] += w
                            else:
                                c1[oi] += w        # pad value is 1.0
        W1_of[i] = W1
        c1_of[i] = c1

    # K-layout: groups concatenated tightly; only the final tile is padded
    K_start = {}
    off = 0
    for g in GROUPS:
        for b in g:
            K_start[b] = off
            off += n_of[b]
    assert off == _NF_TRUE
    border = [b for g in GROUPS for b in g]       # h block order
    hpos = {b: j * 16 for j, b in enumerate(border)}

    W1p = np.zeros((64, NF))
    c1p = np.zeros((NF,))
    W2p = np.zeros((NF, 128))
    b2p = np.zeros((128,))
    for b in range(8):
        s, n, hp = K_start[b], n_of[b], hpos[b]
        W1p[:, s:s + n] = W1_of[b]
        c1p[s:s + n] = c1_of[b]
        W2p[s:s + n, hp:hp + 16] = np.asarray(p[f"b{b}_w1"], np.float64).T
        b2p[hp:hp + 16] = np.asarray(p[f"b{b}_b1"], np.float64)

    Wb = np.zeros((128, 64))
    bb = np.zeros((64,))
    for b in range(8):
        hp = hpos[b]
        Wb[hp:hp + 16, 8 * b:8 * b + 8] = np.asarray(p[f"b{b}_w2"], np.float64).T
        bb[8 * b:8 * b + 8] = np.asarray(p[f"b{b}_b2"], np.float64)
    fc_w1 = np.asarray(p["fc_w1"], np.float64)
    fc_b1 = np.asarray(p["fc_b1"], np.float64)
    W3 = Wb @ fc_w1.T
    b3 = bb @ fc_w1.T + fc_b1
    fc_w2 = np.asarray(p["fc_w2"], np.float64)
    fc_b2 = np.asarray(p["fc_b2"], np.float64)
    fc_w3 = np.asarray(p["fc_w3"], np.float64)
    fc_b3 = np.asarray(p["fc_b3"], np.float64)
    W4 = np.zeros((64, 17)); W4[:, :16] = fc_w2.T
    b4 = np.zeros((17,)); b4[:16] = fc_b2; b4[16] = 1.0
    W5 = np.zeros((17, 1)); W5[:16, 0] = fc_w3[0]; W5[16, 0] = fc_b3[0]

    # Vector-assigned tiles use relu + decomposition:
    #   Lrelu(v) = 0.01*v + 0.99*relu(v)
    # so their W2 rows are scaled by 0.99 and the 0.01*v path (affine in x)
    # folds into W12 / b2f.  Scalar-assigned tiles compute Lrelu exactly on
    # the ACT engine, so their W2 rows stay unscaled and contribute nothing
    # to the fold.
    vmask = np.zeros((NF, 1))
    for t in range(KT):
        if t in VEC_TILES:
            vmask[128 * t:128 * (t + 1)] = 1.0
    W2s = np.where(vmask > 0, (1.0 - LRELU_NEG) * W2p, W2p)
    W12 = LRELU_NEG * ((W1p * vmask[:, 0][None, :]) @ W2p)
    b2f = LRELU_NEG * ((c1p * vmask[:, 0]) @ W2p) + b2p

    f32 = np.float32
    f16 = np.float16
    dev = {}
    # step-1 weight tiles: full K=128 (input is [x; 1; zeros]); row 64
    # carries the y-bias c1 so the PSUM result is already biased.  Packed
    # in unit order so the first DMA chunk covers the first units.
    w1 = np.zeros((128, KT, 128), f16)
    for i, t in enumerate(U_SEQ):
        w1[0:64, i, :] = W1p[:, 128 * t:128 * (t + 1)]
        w1[64, i, :] = c1p[128 * t:128 * (t + 1)]
    dev["w1"] = w1
    w2 = np.zeros((128, KT, 128), f16)
    for t in range(KT):
        w2[:, t, :] = W2s[128 * t:128 * (t + 1), :]
    dev["w2"] = w2
    dev["w12"] = W12.astype(f16)
    dev["b2f"] = b2f.reshape(128, 1).astype(f32)
    dev["w3"] = W3.astype(f16)
    dev["b3"] = b3.reshape(64, 1).astype(f32)
    dev["w4"] = W4.astype(f16)
    dev["b4"] = b4.reshape(17, 1).astype(f32)
    dev["w5"] = W5.astype(f16)
    return dev


# ---------------------------------------------------------------- device IR
def _build_nc(n_pairs=N_PAIRS):
    import concourse.mybir as mybir
    import concourse.tile as tile
    from concourse import bacc
    from contextlib import ExitStack

    dt = mybir.dt
    AF = mybir.ActivationFunctionType
    f32 = dt.float32
    f16 = dt.float16
    bc = n_pairs * PAIR_N

    nc = bacc.Bacc("TRN2", target_bir_lowering=False, debug=False,
                   num_devices=N_CORES)

    xx_d = nc.dram_tensor("xx", [128, bc], f16, kind="ExternalInput")
    w1_d = nc.dram_tensor("w1", [128, KT, 128], f16, kind="ExternalInput")
    w2_d = nc.dram_tensor("w2", [128, KT, 128], f16, kind="ExternalInput")
    w12_d = nc.dram_tensor("w12", [64, 128], f16, kind="ExternalInput")
    b2f_d = nc.dram_tensor("b2f", [128, 1], f32, kind="ExternalInput")
    w3_d = nc.dram_tensor("w3", [128, 64], f16, kind="ExternalInput")
    b3_d = nc.dram_tensor("b3", [64, 1], f32, kind="ExternalInput")
    w4_d = nc.dram_tensor("w4", [64, 17], f16, kind="ExternalInput")
    b4_d = nc.dram_tensor("b4", [17, 1], f32, kind="ExternalInput")
    w5_d = nc.dram_tensor("w5", [17, 1], f16, kind="ExternalInput")
    o_d = nc.dram_tensor("o", [1, bc], f32, kind="ExternalOutput")

    with tile.TileContext(nc) as tc, ExitStack() as ctx:
        wpool = ctx.enter_context(tc.tile_pool(name="wpool", bufs=1))
        xpool = ctx.enter_context(tc.tile_pool(name="xpool", bufs=3))
        ypool = ctx.enter_context(tc.tile_pool(name="ypool", bufs=KT + 3))
        spool = ctx.enter_context(tc.tile_pool(name="spool", bufs=2))
        ps1p = ctx.enter_context(tc.tile_pool(name="ps1p", bufs=3, space="PSUM"))
        ps2p = ctx.enter_context(tc.tile_pool(name="ps2p", bufs=1, space="PSUM"))

        # pair-0 input first so compute can start while the rest streams in
        xx_first = xpool.tile([128, PAIR_N], f16, tag="xx", name="xx_first")
        nc.sync.dma_start(xx_first[:], xx_d[:, 0:PAIR_N])
        # weight DMAs spread across the DMA-capable queues: gpsimd carries
        # the step-1-critical tensors (w1 split so the first units' weights
        # land early), scalar the later-needed step-2 weights.
        w1a_t = wpool.tile([128, 8, 128], f16, name="w1a")
        nc.gpsimd.dma_start(w1a_t[:], w1_d[:, 0:8, :])
        w1b_t = wpool.tile([128, KT - 8, 128], f16, name="w1b")
        nc.gpsimd.dma_start(w1b_t[:], w1_d[:, 8:KT, :])
        b2f_t = wpool.tile([128, 1], f32)
        nc.gpsimd.dma_start(b2f_t[:], b2f_d[:])
        w2_t = wpool.tile([128, KT, 128], f16)
        nc.scalar.dma_start(w2_t[:], w2_d[:])
        w12_t = wpool.tile([64, 128], f16)
        nc.gpsimd.dma_start(w12_t[:], w12_d[:])
        w3_t = wpool.tile([128, 64], f16)
        nc.gpsimd.dma_start(w3_t[:], w3_d[:])
        b3_t = wpool.tile([64, 1], f32)
        nc.gpsimd.dma_start(b3_t[:], b3_d[:])
        w4_t = wpool.tile([64, 17], f16)
        nc.gpsimd.dma_start(w4_t[:], w4_d[:])
        b4_t = wpool.tile([17, 1], f32)
        nc.gpsimd.dma_start(b4_t[:], b4_d[:])
        w5_t = wpool.tile([17, 1], f16)
        nc.gpsimd.dma_start(w5_t[:], w5_d[:])

        def w1s(t):
            return w1a_t[:, t, :] if t < 8 else w1b_t[:, t - 8, :]

        # PE warm-up: ~8us of dummy back-to-back K=128 matmuls on a zeroed
        # tile, bridging the startup DMA window so the HAM clock-gate is
        # open (2.4 GHz) when real work starts.
        wu_t = wpool.tile([128, 640], f16, name="wu")
        nc.vector.memset(wu_t[:], 0.0)
        wups = ps1p.tile([128, CHUNK], f32, tag="ps1", name="wups")
        for i in range(19):
            nc.tensor.matmul(wups[:], wu_t[:, 0:128], wu_t[:, 128:640],
                             start=True, stop=True)

        def make_tail_stages(p, ps2):
            """The per-pair serial tail (h -> g1 -> g2 -> out), popped at
            fixed units of the NEXT pair so the cross-engine latencies
            never head-of-line-block the in-order PE queue."""
            st = {}

            def s0():
                st["h"] = spool.tile([128, PAIR_N], f16, tag="h", name=f"h_{p}")
                nc.scalar.activation(st["h"][:], ps2[:], AF.Lrelu,
                                     bias=b2f_t[:, 0:1], alpha=LRELU_NEG)

            def s1():
                st["g1ps"] = ps1p.tile([64, PAIR_N], f32, tag="ps1",
                                       name=f"g1ps_{p}")
                for h in range(2):
                    sl = slice(h * CHUNK, (h + 1) * CHUNK)
                    nc.tensor.matmul(st["g1ps"][:, sl], w3_t[:],
                                     st["h"][:, sl], start=True, stop=True)

            def s2():
                st["g1"] = spool.tile([64, PAIR_N], f16, tag="g1",
                                      name=f"g1_{p}")
                nc.scalar.activation(st["g1"][:], st["g1ps"][:], AF.Lrelu,
                                     bias=b3_t[:, 0:1], alpha=LRELU_NEG)

            def s3():
                st["g2ps"] = ps1p.tile([17, PAIR_N], f32, tag="ps1",
                                       name=f"g2ps_{p}")
                for h in range(2):
                    sl = slice(h * CHUNK, (h + 1) * CHUNK)
                    nc.tensor.matmul(st["g2ps"][:, sl], w4_t[:],
                                     st["g1"][:, sl], start=True, stop=True)

            def s4():
                st["g2"] = spool.tile([17, PAIR_N], f16, tag="g2",
                                      name=f"g2_{p}")
                nc.scalar.activation(st["g2"][:], st["g2ps"][:], AF.Lrelu,
                                     bias=b4_t[:, 0:1], alpha=LRELU_NEG)

            def s5():
                st["ops"] = ps1p.tile([1, PAIR_N], f32, tag="ps1",
                                      name=f"ops_{p}")
                for h in range(2):
                    sl = slice(h * CHUNK, (h + 1) * CHUNK)
                    nc.tensor.matmul(st["ops"][:, sl], w5_t[:],
                                     st["g2"][:, sl], start=True, stop=True)

            def s6():
                o_t = spool.tile([1, PAIR_N], f32, tag="o", name=f"o_{p}")
                nc.vector.tensor_copy(o_t[:], st["ops"][:])
                nc.sync.dma_start(o_d[:, p * PAIR_N:(p + 1) * PAIR_N], o_t[:])

            return [s0, s1, s2, s3, s4, s5, s6]

        tail_stages = []
        # tail stage -> unit of the NEXT pair it pops at.  Units 0-3 are
        # reserved for the previous pair's carried chain steps, so the
        # h ACT (stage 0) pops at unit 4 and the rest follow.
        TAIL_UNIT = {0: 4, 1: 8, 2: 10, 3: 12, 4: 14, 5: 16, 6: 18}

        def make_chain_emitter(ps2, ytiles, rstate, emit_unit):
            def emit(now_unit, max_rounds):
                """Emit up to max_rounds rounds of the ROUND_PLAN (a round
                is ready once all its tiles' evacuations were emitted >= 3
                units ago), chunk-major: all of a round's h0 chunks
                back-to-back, then its h1 chunks, so up to four 32-column
                tile positions stream concurrently."""
                for _ in range(max_rounds):
                    if rstate[0] >= len(ROUND_PLAN):
                        return
                    rnd = ROUND_PLAN[rstate[0]]
                    if not all(emit_unit.get(t, 99) <= now_unit - 3
                               for (_, _, t, _) in rnd):
                        return
                    for h in range(2):
                        sl = slice(h * CHUNK, (h + 1) * CHUNK)
                        for (c0, cw, t, stop) in rnd:
                            nc.tensor.matmul(
                                ps2[c0:c0 + cw, sl],
                                w2_t[:, t, c0:c0 + cw],
                                ytiles[t][:, sl], start=False, stop=stop,
                                tile_position=(0, c0),
                                skip_group_check=True)
                    rstate[0] += 1

            def remaining():
                return rstate[0] < len(ROUND_PLAN)

            return emit, remaining

        carry_emit = None        # leftover chain steps of the previous pair
        carry_rem = None

        xx_next = xx_first
        for p in range(n_pairs):
            xx_t = xx_next
            stages = tail_stages
            tail_stages = []

            ps2 = ps2p.tile([128, PAIR_N], f32, tag="ps2", name=f"ps2_{p}")
            ytiles = [None] * KT
            emit_unit = {}
            rstate = [0]
            emit_chain_rounds, chain_remaining = make_chain_emitter(
                ps2, ytiles, rstate, emit_unit)

            stage_i = 0
            for u, t in enumerate(U_SEQ):
                while (stage_i < len(stages)
                       and TAIL_UNIT[stage_i] <= u):
                    stages[stage_i]()
                    stage_i += 1
                if u <= 3 and carry_emit is not None:
                    # previous pair's carried chain steps, interleaved into
                    # this pair's first units so the PE queue never parks a
                    # whole flush (which would starve the evac engines at
                    # the boundary); force-drained by the end of unit 3
                    # because the h ACT of that pair pops at unit 4.
                    carry_emit(99, 2 if u < 3 else 12)
                    if u == 3:
                        assert not carry_rem()
                        carry_emit = carry_rem = None
                if p + 1 < n_pairs and u == 2:
                    xx_next = xpool.tile([128, PAIR_N], f16, tag="xx",
                                         name=f"xx_{p + 1}")
                    nc.sync.dma_start(
                        xx_next[:],
                        xx_d[:, (p + 1) * PAIR_N:(p + 2) * PAIR_N])

                psU = ps1p.tile([128, PAIR_N], f32, tag="ps1",
                                name=f"psU_{p}_{u}")
                for h in range(2):
                    sl = slice(h * CHUNK, (h + 1) * CHUNK)
                    nc.tensor.matmul(psU[:, sl], w1s(u), xx_t[:, sl],
                                     start=True, stop=True)
                y_t = ypool.tile([128, PAIR_N], f16, tag="y",
                                 name=f"y_{p}_{t}")
                if u in _VEC_POS:
                    nc.vector.tensor_scalar_max(y_t[:], psU[:], 0.0)
                else:
                    nc.scalar.activation(y_t[:], psU[:], AF.Lrelu,
                                         alpha=LRELU_NEG)
                ytiles[t] = y_t
                emit_unit[t] = u

                if u == 6:
                    # first write of ps2: the 0.01*y fold path (vector tiles
                    # only).  Emitted late enough that the previous pair's
                    # h-act (popped at unit 4) has released the bank.
                    for h in range(2):
                        sl = slice(h * CHUNK, (h + 1) * CHUNK)
                        nc.tensor.matmul(
                            ps2[:, sl], w12_t[:], xx_t[0:64, sl],
                            start=True, stop=False, tile_position=(0, 0),
                            skip_group_check=True)
                if u in (7, 10, 14, 18, 21, 22):
                    # batched chain points: fewer tile-position switches on
                    # the PE (each switch costs ~106ns of un-hidden
                    # LDWEIGHTS), and each point's round is 3-4-way packed
                    # thanks to the round-ordered unit sequence.
                    emit_chain_rounds(u, 1)

            # ---- the rest of this pair's chain steps are carried into the
            # next pair's units 0-3 (or flushed below for the last pair).
            while stage_i < len(stages):
                stages[stage_i]()
                stage_i += 1
            carry_emit, carry_rem = emit_chain_rounds, chain_remaining
            tail_stages = make_tail_stages(p, ps2)

        while carry_rem():
            carry_emit(99, 1)
        for st in tail_stages:
            st()

    nc.compile()
    return nc


# ---------------------------------------------------------------- execution
_NC_CACHE = {}
LAST_RESULT = None


def _prep_inputs(inputs):
    board = np.ascontiguousarray(np.asarray(inputs["board"], np.float32))
    x = board.reshape(B_TOTAL, 64)
    dev = _fold_params(inputs)
    ones = np.ones((1, BC), np.float16)
    zeros = np.zeros((63, BC), np.float16)
    in_maps = []
    for c in range(N_CORES):
        xc = np.ascontiguousarray(x[c * BC:(c + 1) * BC].T)      # [64, BC]
        m = dict(dev)
        m["xx"] = np.ascontiguousarray(
            np.vstack([xc.astype(np.float16), ones, zeros]))     # [128, BC]
        in_maps.append(m)
    return in_maps


def kernel(**inputs):
    global LAST_RESULT
    from concourse.bass_utils import run_bass_kernel_spmd

    if "nc" not in _NC_CACHE:
        _NC_CACHE["nc"] = _build_nc()
    nc = _NC_CACHE["nc"]

    in_maps = _prep_inputs(inputs)
    res = run_bass_kernel_spmd(nc, in_maps, core_ids=list(range(N_CORES)))
    LAST_RESULT = res
    out = np.concatenate([r["o"].reshape(-1) for r in res.results])
    return out.reshape(B_TOTAL, 1).astype(np.float32)
